# revision 27
# baseline (speedup 1.0000x reference)
"""DistanceTransformLoss on 8 Trainium2 NeuronCores (Bass/Tile).

loss = BCEWithLogits(predictions, targets).mean()
       + sqrt( sum(pen) / max(count(pen != 0), 1) ),
  pen = (sigmoid(pred) > 0.5) * grassfire_dist_H(targets)

Distance via matmul in natural layout (h on partitions):
  S[i,w] = sum_j q^|i-j| * t[j,w] with q = 2^-4 (banded bf16 matmuls);
  S = 2^(-4*D) * rho, rho in [1, 2.14), so the exact column distance D
  is recovered from the f32 exponent field: d16 = int16(-hi/512 + 32),
  hi = high halfword of S; the value lands in (D - 0.25, D + 0.25].

v4 engine assignment (the accumulate-capable DVE ops all run in 1x
perf mode; only plain tensor_scalar (4x) and tensor_tensor (2x) are
fast, so sums go to ACT-accumulate and PE ones-matmuls):
  - DMA (one SWDGE queue): p and t cast f32->bf16 during the DMA,
    one 2MB-read transfer per tensor per half-image unit
  - ACT: e = Exp(p); softplus Ln(e+1) in-place accumulate -> sum_sp;
    decode of S chunks 0,2 per unit
  - DVE: m = (p>0) TS 4x; pt = p*t TT 2x; md = m*d TT 2x;
    c01 = (md>0) TS 4x; decode of S chunks 1,3 per unit
  - PE:  banded S-matmuls + ones-matmul column sums of pt/md/c01 into
    three persistent [1,512] psum accumulators
Host (f64): bce = (sum_sp - sum_pt)/NEL; border = sum_md/max(cnt,1);
  loss = bce + sqrt(border).

Sharding: data-parallel over batch N (32 images -> 4 per core).
"""
import sys

if "/opt/trn_rl_repo" not in sys.path:
    sys.path.insert(0, "/opt/trn_rl_repo")

import numpy as np
from contextlib import ExitStack

import concourse.bass as bass
import concourse.bacc as bacc
import concourse.tile as tile
from concourse import mybir
from concourse.ap import AP
from concourse.bass_utils import run_bass_kernel_spmd
from concourse.hw_specs import get_activation_tables

N_CORES = 8
N_PER_CORE = 4          # 32 images / 8 cores
H = 1024
W = 1024
HC = 4                  # 128-row chunks per half-image unit
NU = 2 * N_PER_CORE     # pipeline units (half-images) per core

F32 = mybir.dt.float32
F16 = mybir.dt.float16
BF16 = mybir.dt.bfloat16
I16 = mybir.dt.int16

# acc layout: [128, 11] f32 columns:
#   [0:8)  softplus sums per unit (ACT accumulate)
#   [8]    sum p*t   (row 0; psum cache-reduce)
#   [9]    sum m*d   (row 0)
#   [10]   count     (row 0)
ACC_COLS = 11

_CACHED_NC = None


def _flat(ap):
    """Flatten the free dims of a contiguous [128, ...] AP to [128, F]."""
    (pstep, pcount) = ap.ap[0]
    f = 1
    for (_, c) in ap.ap[1:]:
        f *= c
    return AP(ap.tensor, ap.offset, [[pstep, pcount], [1, f]])


def _k_blocks():
    """The three constant kernel blocks [j, i] in bf16, q = 2^-4.

    KD[j,i] = q^|i-j|, KU[j,i] = q^(128+i-j), KL[j,i] = q^(128+j-i).
    All entries are exact powers of two (zero beyond distance 31), so
    S = sum_j K[i,j] t[j] = 2^(-4D) * rho with rho in [1, 2.14) and the
    distance D is recovered exactly from the f32 exponent field of S.
    Host-precast to bf16 so they load over the idle HWDGE queue.
    """
    import ml_dtypes
    j = np.arange(128, dtype=np.float64)[:, None]
    i = np.arange(128, dtype=np.float64)[None, :]
    out = []
    for dmat in (np.abs(i - j), 128.0 + i - j, 128.0 + j - i):
        k = np.where(dmat <= 31, np.power(2.0, -4.0 * dmat), 0.0)
        out.append(np.ascontiguousarray(k.astype(ml_dtypes.bfloat16)))
    return out


def _unit_src(ext, n, h):
    """DRAM AP for one half-image in chunk-major [128, HC, W] order."""
    off = n * H * W + h * (H // 2) * W
    return AP(ext.tensor, off, [[W, 128], [128 * W, HC], [1, W]])


def _build_nc():
    nc = bacc.Bacc("TRN2", target_bir_lowering=False, debug=False,
                   enable_asserts=False)
    t_ext = nc.dram_tensor("targets", [N_PER_CORE, H, W], F32,
                           kind="ExternalInput").ap()
    p_ext = nc.dram_tensor("predictions", [N_PER_CORE, H, W], F32,
                           kind="ExternalInput").ap()
    kd_ext = nc.dram_tensor("kd", [128, 128], BF16, kind="ExternalInput").ap()
    ku_ext = nc.dram_tensor("ku", [128, 128], BF16, kind="ExternalInput").ap()
    kl_ext = nc.dram_tensor("kl", [128, 128], BF16, kind="ExternalInput").ap()
    acc_ext = nc.dram_tensor("acc", [128, ACC_COLS], F32,
                             kind="ExternalOutput").ap()

    with tile.TileContext(nc) as tc, ExitStack() as ctx:
        const_pool = ctx.enter_context(tc.tile_pool(name="const", bufs=1))
        p_pool = ctx.enter_context(tc.tile_pool(name="p16", bufs=5))
        t_pool = ctx.enter_context(tc.tile_pool(name="t16", bufs=5))
        e_pool = ctx.enter_context(tc.tile_pool(name="e", bufs=2))
        m_pool = ctx.enter_context(tc.tile_pool(name="m", bufs=3))
        d_pool = ctx.enter_context(tc.tile_pool(name="d16", bufs=2))
        md_pool = ctx.enter_context(tc.tile_pool(name="md", bufs=2))
        pt_pool = ctx.enter_context(tc.tile_pool(name="pt", bufs=3))
        c_pool = ctx.enter_context(tc.tile_pool(name="c01", bufs=2))
        acc_pool = ctx.enter_context(tc.tile_pool(name="acc", bufs=1))
        ps_pool = ctx.enter_context(tc.tile_pool(name="ps", bufs=3,
                                                 space="PSUM"))
        psacc_pool = ctx.enter_context(tc.tile_pool(name="psacc", bufs=1,
                                                    space="PSUM"))

        # Pre-load the act table containing BOTH Exp and Ln.
        tables = list(get_activation_tables(nc.m.arch).items())
        set_id = next(i for i, (_, fns) in enumerate(tables)
                      if mybir.ActivationFunctionType.Exp in fns
                      and mybir.ActivationFunctionType.Ln in fns)
        nc.scalar.add_instruction(mybir.InstLoadActFuncSet(
            name=nc.get_next_instruction_name(),
            act_func_set_id=set_id, ins=[], outs=[]))

        # constants: kernel blocks (host-precast bf16, HWDGE queue so the
        # gpsimd SWDGE queue starts on the first input transfer at t=0)
        kd = const_pool.tile([128, 128], BF16, tag="kd")
        ku = const_pool.tile([128, 128], BF16, tag="ku")
        kl = const_pool.tile([128, 128], BF16, tag="kl")
        nc.sync.dma_start(kd[:], kd_ext)
        nc.sync.dma_start(ku[:], ku_ext)
        nc.sync.dma_start(kl[:], kl_ext)

        accs = acc_pool.tile([128, ACC_COLS], F32)
        nc.vector.memset(accs[:], 0.0)
        ones_f16 = const_pool.tile([128, 1], F16, tag="ones_f16")
        nc.gpsimd.memset(ones_f16[:], 1.0)

        # persistent PE reduction accumulators in one psum bank
        # (matmul out base partition must be 0/32/64)
        ps_all = psacc_pool.tile([65, 512], F32, tag="ps_all")
        ps_pt = ps_all[0:1, :]
        ps_md = ps_all[32:33, :]
        ps_c = ps_all[64:65, :]

        st = [dict() for _ in range(NU)]

        def phase_a(u):
            """SWDGE cast loads: one 2MB-read transfer per tensor."""
            n, h = u // 2, u % 2
            t16 = t_pool.tile([128, HC, W], BF16, tag="t16")
            p16 = p_pool.tile([128, HC, W], BF16, tag="p16")
            if u == 0:
                # head: half-granularity so Exp/matmuls start ~4us sooner
                psrc, tsrc = _unit_src(p_ext, n, h), _unit_src(t_ext, n, h)
                for hf in range(2):
                    cs = slice(2 * hf, 2 * hf + 2)
                    ph = AP(psrc.tensor, psrc.offset + hf * 2 * 128 * W,
                            [[W, 128], [128 * W, 2], [1, W]])
                    th = AP(tsrc.tensor, tsrc.offset + hf * 2 * 128 * W,
                            [[W, 128], [128 * W, 2], [1, W]])
                    nc.gpsimd.dma_start(p16[:, cs, :], ph)
                    nc.gpsimd.dma_start(t16[:, cs, :], th)
            elif u >= NU - 2 or u == 1:
                # head: Exp can start early; tail: BCE chain finishes early
                nc.gpsimd.dma_start(p16[:], _unit_src(p_ext, n, h))
                nc.gpsimd.dma_start(t16[:], _unit_src(t_ext, n, h))
            else:
                nc.gpsimd.dma_start(t16[:], _unit_src(t_ext, n, h))
                nc.gpsimd.dma_start(p16[:], _unit_src(p_ext, n, h))
            st[u]["t16"] = t16
            st[u]["p16"] = p16

        def phase_e(u):
            """ACT: e = Exp(p).  DVE: m = (p > 0) in 4x mode."""
            p16 = st[u]["p16"]
            e = e_pool.tile([128, HC * W], F16, tag="e")
            if u == 0:
                for hf in range(2):
                    sg = slice(hf * 2 * W, (hf + 1) * 2 * W)
                    nc.scalar.activation(e[:, sg], _flat(p16[:])[:, sg],
                                         mybir.ActivationFunctionType.Exp)
            else:
                nc.scalar.activation(e[:], _flat(p16[:]),
                                     mybir.ActivationFunctionType.Exp)
            m = m_pool.tile([128, HC * W], BF16, tag="m")
            nc.vector.tensor_scalar(m[:], _flat(p16[:]), 0.0, None,
                                    mybir.AluOpType.is_gt)
            st[u]["e"] = e
            st[u]["m"] = m

        def phase_sums(u, half=None, reduces=None):
            """PE ones-matmul column sums of pt/md/c01 (prev unit).
            `reduces`: accs columns to drain each accumulator into right
            after its stop matmul (tail interleaving)."""
            first, last = (u == 0), (u == NU - 1)
            pt, md, c01 = st[u]["pt"], st[u]["md"], st[u]["c01"]
            segs = range(8) if half is None else range(4 * half, 4 * half + 4)
            for dst, src, col in ((ps_pt, pt, 8), (ps_md, md, 9),
                                  (ps_c, c01, 10)):
                for s in segs:
                    ws = slice(s * 512, (s + 1) * 512)
                    nc.tensor.matmul(dst, ones_f16[:], src[:, ws],
                                     start=(first and s == 0),
                                     stop=(last and s == 7))
                if reduces is not None and 7 in segs:
                    nc.vector.tensor_scalar(
                        dst, dst, 1.0, 0.0, mybir.AluOpType.mult,
                        mybir.AluOpType.add,
                        accum_out=accs[0:1, col:col + 1])

        def phase_pt(u):
            """DVE p*t (ready early, emitted before psum-dependent work)."""
            p16, t16 = st[u]["p16"], st[u]["t16"]
            pt = pt_pool.tile([128, HC * W], F16, tag="pt")
            nc.vector.tensor_tensor(pt[:], _flat(p16[:]), _flat(t16[:]),
                                    mybir.AluOpType.mult)
            st[u]["pt"] = pt

        def phase_ln(u):
            """ACT softplus accumulate (block end, after the decodes)."""
            e = st[u]["e"]
            nc.scalar.activation(e[:], e[:],
                                 mybir.ActivationFunctionType.Ln,
                                 bias=1.0, accum_out=accs[:, u:u + 1])

        def _mm_chunk(u, j, s_tile):
            """Accumulating banded matmuls for chunk j of unit u."""
            n, h = u // 2, u % 2
            c = h * HC + j
            kl_ = [(kd, c)]
            if c > 0:
                kl_.append((ku, c - 1))
            if c < 2 * HC - 1:
                kl_.append((kl, c + 1))
            for q, (kmat, srcc) in enumerate(kl_):
                src_u = n * 2 + srcc // HC
                src_j = srcc % HC
                t_src = st[src_u]["t16"]
                for wh in range(2):
                    ws = slice(wh * 512, (wh + 1) * 512)
                    nc.tensor.matmul(s_tile[:, ws], kmat[:],
                                     t_src[:, src_j, ws],
                                     start=(q == 0),
                                     stop=(q == len(kl_) - 1))

        def _decode_chunk(d16, j, s_tile, on_act):
            """d16[:,j,:] = int16(-hi/512 + 32), hi = S high halfword."""
            b = s_tile[:].bitcast(I16)
            hi = AP(b.tensor, b.offset + 1, [list(b.ap[0]), [2, W]])
            if on_act:
                nc.scalar.activation(d16[:, j, :], hi,
                                     mybir.ActivationFunctionType.Copy,
                                     bias=32.0, scale=-1.0 / 512.0)
            else:
                nc.vector.tensor_scalar(d16[:, j, :], hi, -1.0 / 512.0,
                                        32.0, mybir.AluOpType.mult,
                                        mybir.AluOpType.add)

        def phase_b(u):
            """Banded matmuls -> S psum -> integer distance.
            Decode alternates DVE (chunks 0,2) / ACT (chunks 1,3) in psum
            production order so tiles free steadily for the next unit."""
            d16 = d_pool.tile([128, HC, W], I16, tag="d")
            for j in range(HC):
                s_tile = ps_pool.tile([128, W], F32, tag="s",
                                      name=f"s_{u}_{j}")
                _mm_chunk(u, j, s_tile)
                _decode_chunk(d16, j, s_tile, on_act=(j % 2 == 1))
            st[u]["d16"] = d16

        def phase_md(u, half=None):
            """DVE fused-mask product and count indicator (lagged one
            block so the decodes are always complete)."""
            d16, m = st[u]["d16"], st[u]["m"]
            if "md" not in st[u]:
                st[u]["md"] = md_pool.tile([128, HC * W], F16, tag="md",
                                           name=f"md_{u}")
                st[u]["c01"] = c_pool.tile([128, HC * W], F16, tag="c01",
                                           name=f"c01_{u}")
            md, c01 = st[u]["md"], st[u]["c01"]
            segs = [slice(0, HC * W)] if half is None else [
                slice(half * HC * W // 2, (half + 1) * HC * W // 2)]
            for seg in segs:
                nc.vector.tensor_tensor(md[:, seg], m[:, seg],
                                        _flat(d16[:])[:, seg],
                                        mybir.AluOpType.mult)
                nc.vector.tensor_scalar(c01[:, seg], md[:, seg], 0.0, None,
                                        mybir.AluOpType.is_gt)

        # software-pipelined emission, per-engine queue order per block k:
        #   gpsimd: loads(k+2)
        #   ACT: E(k+1), dec1(k), dec3(k), Ln(k)
        #   DVE: m(k+1), pt(k), dec0(k), dec2(k), md(k-1), c01(k-1)
        #   PE: banded(k), sums(k-2)
        phase_a(0)
        phase_a(1)
        phase_a(2)
        phase_e(0)
        for k in range(NU):
            if k + 3 < NU:
                phase_a(k + 3)
            if k + 1 < NU:
                phase_e(k + 1)
            phase_pt(k)
            phase_b(k)
            if k >= 2:
                phase_sums(k - 2)
            if k >= 1:
                phase_md(k - 1)
            phase_ln(k)
        # tail: finish the last unit at half granularity to shorten the
        # critical chain (PE runs sums while DVE products finish); the
        # psum accumulators drain into acc row 0 right after their stop
        # matmuls, interleaved with the remaining PE sums
        phase_md(NU - 1, half=0)
        phase_sums(NU - 2)
        phase_md(NU - 1, half=1)
        # softplus columns are final once Ln(NU-1) has accumulated
        nc.sync.dma_start(acc_ext[:, 0:8], accs[:, 0:8])
        phase_sums(NU - 1, half=0)
        phase_sums(NU - 1, half=1, reduces=True)

        nc.sync.dma_start(acc_ext[:, 8:11], accs[:, 8:11])

    nc.compile()
    return nc


def _get_nc():
    global _CACHED_NC
    if _CACHED_NC is None:
        _CACHED_NC = _build_nc()
    return _CACHED_NC


def _run(predictions, targets, trace=False, **trace_kwargs):
    """Run the SPMD kernel; returns (loss_scalar, BassKernelResults)."""
    p = np.ascontiguousarray(
        np.asarray(predictions, dtype=np.float32).reshape(32, H, W))
    t = np.ascontiguousarray(
        np.asarray(targets, dtype=np.float32).reshape(32, H, W))
    kd, ku, kl = _k_blocks()

    in_maps = []
    for c in range(N_CORES):
        sl = slice(c * N_PER_CORE, (c + 1) * N_PER_CORE)
        in_maps.append({
            "predictions": np.ascontiguousarray(p[sl]),
            "targets": np.ascontiguousarray(t[sl]),
            "kd": kd, "ku": ku, "kl": kl,
        })

    nc = _get_nc()
    res = run_bass_kernel_spmd(nc, in_maps, list(range(N_CORES)),
                               trace=trace, **trace_kwargs)

    sum_sp = sum_pt = sum_md = cnt = 0.0
    for c in range(N_CORES):
        acc = np.asarray(res.results[c]["acc"], dtype=np.float64)
        sum_sp += acc[:, 0:8].sum()
        sum_pt += acc[0, 8]
        sum_md += acc[0, 9]
        cnt += acc[0, 10]

    n_elem = 32.0 * H * W
    bce = (sum_sp - sum_pt) / n_elem
    border = 0.0 if sum_md == 0.0 else sum_md / max(cnt, 1.0)
    loss = bce + np.sqrt(max(border, 0.0))
    return np.float32(loss), res


def kernel(predictions, targets):
    loss, _ = _run(predictions, targets)
    return np.asarray(loss, dtype=np.float32)


# revision 30
# speedup vs baseline: 1.0669x; 1.0669x over previous
"""DistanceTransformLoss on 8 Trainium2 NeuronCores (Bass/Tile).

loss = BCEWithLogits(predictions, targets).mean()
       + sqrt( sum(pen) / max(count(pen != 0), 1) ),
  pen = (sigmoid(pred) > 0.5) * grassfire_dist_H(targets)

Distance via matmul in natural layout (h on partitions):
  S[i,w] = sum_j q^|i-j| * t[j,w] with q = 2^-4 (banded bf16 matmuls);
  S = 2^(-4*D) * rho, rho in [1, 2.14), so the exact column distance D
  is recovered from the f32 exponent field: d16 = int16(-hi/512 + 32),
  hi = high halfword of S; the value lands in (D - 0.25, D + 0.25].

v4 engine assignment (the accumulate-capable DVE ops all run in 1x
perf mode; only plain tensor_scalar (4x) and tensor_tensor (2x) are
fast, so sums go to ACT-accumulate and PE ones-matmuls):
  - DMA (one SWDGE queue): p and t cast f32->bf16 during the DMA,
    one 2MB-read transfer per tensor per half-image unit
  - ACT: e = Exp(p); softplus Ln(e+1) in-place accumulate -> sum_sp;
    decode of S chunks 0,2 per unit
  - DVE: m = (p>0) TS 4x; pt = p*t TT 2x; md = m*d TT 2x;
    c01 = (md>0) TS 4x; decode of S chunks 1,3 per unit
  - PE:  banded S-matmuls + ones-matmul column sums of pt/md/c01 into
    three persistent [1,512] psum accumulators
Host (f64): bce = (sum_sp - sum_pt)/NEL; border = sum_md/max(cnt,1);
  loss = bce + sqrt(border).

Sharding: data-parallel over batch N (32 images -> 4 per core).
"""
import sys

if "/opt/trn_rl_repo" not in sys.path:
    sys.path.insert(0, "/opt/trn_rl_repo")

import numpy as np
from contextlib import ExitStack

import concourse.bass as bass
import concourse.bacc as bacc
import concourse.tile as tile
from concourse import mybir
from concourse.ap import AP
from concourse.bass_utils import run_bass_kernel_spmd
from concourse.hw_specs import get_activation_tables

N_CORES = 8
N_PER_CORE = 4          # 32 images / 8 cores
H = 1024
W = 1024
HC = 4                  # 128-row chunks per half-image unit
NU = 2 * N_PER_CORE     # pipeline units (half-images) per core

F32 = mybir.dt.float32
F16 = mybir.dt.float16
BF16 = mybir.dt.bfloat16
I16 = mybir.dt.int16

# acc layout: [128, 11] f32 columns:
#   [0:8)  softplus sums per unit (ACT accumulate)
#   [8]    sum p*t   (row 0; psum cache-reduce)
#   [9]    sum m*d   (row 0)
#   [10]   count     (row 0)
ACC_COLS = 11

_CACHED_NC = None


def _flat(ap):
    """Flatten the free dims of a contiguous [128, ...] AP to [128, F]."""
    (pstep, pcount) = ap.ap[0]
    f = 1
    for (_, c) in ap.ap[1:]:
        f *= c
    return AP(ap.tensor, ap.offset, [[pstep, pcount], [1, f]])


def _k_blocks():
    """The three constant kernel blocks [j, i] in bf16, q = 2^-4.

    KD[j,i] = q^|i-j|, KU[j,i] = q^(128+i-j), KL[j,i] = q^(128+j-i).
    All entries are exact powers of two (zero beyond distance 31), so
    S = sum_j K[i,j] t[j] = 2^(-4D) * rho with rho in [1, 2.14) and the
    distance D is recovered exactly from the f32 exponent field of S.
    Host-precast to bf16 so they load over the idle HWDGE queue.
    """
    import ml_dtypes
    j = np.arange(128, dtype=np.float64)[:, None]
    i = np.arange(128, dtype=np.float64)[None, :]
    out = []
    for dmat in (np.abs(i - j), 128.0 + i - j, 128.0 + j - i):
        k = np.where(dmat <= 31, np.power(2.0, -4.0 * dmat), 0.0)
        out.append(np.ascontiguousarray(k.astype(ml_dtypes.bfloat16)))
    return out


def _unit_src(ext, n, h):
    """DRAM AP for one half-image in chunk-major [128, HC, W] order."""
    off = n * H * W + h * (H // 2) * W
    return AP(ext.tensor, off, [[W, 128], [128 * W, HC], [1, W]])


def _build_nc():
    nc = bacc.Bacc("TRN2", target_bir_lowering=False, debug=False,
                   enable_asserts=False)
    t_ext = nc.dram_tensor("targets", [N_PER_CORE, H, W], F32,
                           kind="ExternalInput").ap()
    p_ext = nc.dram_tensor("predictions", [N_PER_CORE, H, W], F32,
                           kind="ExternalInput").ap()
    kd_ext = nc.dram_tensor("kd", [128, 128], BF16, kind="ExternalInput").ap()
    ku_ext = nc.dram_tensor("ku", [128, 128], BF16, kind="ExternalInput").ap()
    kl_ext = nc.dram_tensor("kl", [128, 128], BF16, kind="ExternalInput").ap()
    acc_ext = nc.dram_tensor("acc", [128, ACC_COLS], F32,
                             kind="ExternalOutput").ap()

    with tile.TileContext(nc) as tc, ExitStack() as ctx:
        const_pool = ctx.enter_context(tc.tile_pool(name="const", bufs=1))
        p_pool = ctx.enter_context(tc.tile_pool(name="p16", bufs=5))
        t_pool = ctx.enter_context(tc.tile_pool(name="t16", bufs=5))
        e_pool = ctx.enter_context(tc.tile_pool(name="e", bufs=2))
        m_pool = ctx.enter_context(tc.tile_pool(name="m", bufs=3))
        d_pool = ctx.enter_context(tc.tile_pool(name="d16", bufs=2))
        md_pool = ctx.enter_context(tc.tile_pool(name="md", bufs=2))
        pt_pool = ctx.enter_context(tc.tile_pool(name="pt", bufs=3))
        c_pool = ctx.enter_context(tc.tile_pool(name="c01", bufs=2))
        acc_pool = ctx.enter_context(tc.tile_pool(name="acc", bufs=1))
        ps_pool = ctx.enter_context(tc.tile_pool(name="ps", bufs=3,
                                                 space="PSUM"))
        psacc_pool = ctx.enter_context(tc.tile_pool(name="psacc", bufs=1,
                                                    space="PSUM"))

        # Pre-load the act table containing BOTH Exp and Ln.
        tables = list(get_activation_tables(nc.m.arch).items())
        set_id = next(i for i, (_, fns) in enumerate(tables)
                      if mybir.ActivationFunctionType.Exp in fns
                      and mybir.ActivationFunctionType.Ln in fns)
        nc.scalar.add_instruction(mybir.InstLoadActFuncSet(
            name=nc.get_next_instruction_name(),
            act_func_set_id=set_id, ins=[], outs=[]))

        # constants: kernel blocks (host-precast bf16, HWDGE queue so the
        # gpsimd SWDGE queue starts on the first input transfer at t=0)
        kd = const_pool.tile([128, 128], BF16, tag="kd")
        ku = const_pool.tile([128, 128], BF16, tag="ku")
        kl = const_pool.tile([128, 128], BF16, tag="kl")
        nc.sync.dma_start(kd[:], kd_ext)
        nc.sync.dma_start(ku[:], ku_ext)
        nc.sync.dma_start(kl[:], kl_ext)

        accs = acc_pool.tile([128, ACC_COLS], F32)
        nc.vector.memset(accs[:], 0.0)
        ones_f16 = const_pool.tile([128, 1], F16, tag="ones_f16")
        nc.gpsimd.memset(ones_f16[:], 1.0)

        # persistent PE reduction accumulators in one psum bank
        # (matmul out base partition must be 0/32/64)
        ps_all = psacc_pool.tile([65, 512], F32, tag="ps_all")
        ps_pt = ps_all[0:1, :]
        ps_md = ps_all[32:33, :]
        ps_c = ps_all[64:65, :]

        st = [dict() for _ in range(NU)]

        def phase_a(u):
            """SWDGE cast loads: one 2MB-read transfer per tensor."""
            n, h = u // 2, u % 2
            t16 = t_pool.tile([128, HC, W], BF16, tag="t16")
            p16 = p_pool.tile([128, HC, W], BF16, tag="p16")
            if u >= NU - 2 or u <= 1:
                # head: Exp can start early; tail: BCE chain finishes early
                nc.gpsimd.dma_start(p16[:], _unit_src(p_ext, n, h))
                nc.gpsimd.dma_start(t16[:], _unit_src(t_ext, n, h))
            else:
                nc.gpsimd.dma_start(t16[:], _unit_src(t_ext, n, h))
                nc.gpsimd.dma_start(p16[:], _unit_src(p_ext, n, h))
            st[u]["t16"] = t16
            st[u]["p16"] = p16

        def phase_e(u):
            """ACT: e = Exp(p).  DVE: m = (p > 0) in 4x mode."""
            p16 = st[u]["p16"]
            e = e_pool.tile([128, HC * W], F16, tag="e")
            nc.scalar.activation(e[:], _flat(p16[:]),
                                 mybir.ActivationFunctionType.Exp)
            m = m_pool.tile([128, HC * W], BF16, tag="m")
            nc.vector.tensor_scalar(m[:], _flat(p16[:]), 0.0, None,
                                    mybir.AluOpType.is_gt)
            st[u]["e"] = e
            st[u]["m"] = m

        def phase_sums(u, half=None, reduces=None):
            """PE ones-matmul column sums of pt/md/c01 (prev unit).
            `reduces`: accs columns to drain each accumulator into right
            after its stop matmul (tail interleaving)."""
            first, last = (u == 0), (u == NU - 1)
            pt, md, c01 = st[u]["pt"], st[u]["md"], st[u]["c01"]
            segs = range(8) if half is None else range(4 * half, 4 * half + 4)
            for dst, src, col in ((ps_pt, pt, 8), (ps_md, md, 9),
                                  (ps_c, c01, 10)):
                for s in segs:
                    ws = slice(s * 512, (s + 1) * 512)
                    nc.tensor.matmul(dst, ones_f16[:], src[:, ws],
                                     start=(first and s == 0),
                                     stop=(last and s == 7))
                if reduces is not None and 7 in segs:
                    nc.vector.tensor_scalar(
                        dst, dst, 1.0, 0.0, mybir.AluOpType.mult,
                        mybir.AluOpType.add,
                        accum_out=accs[0:1, col:col + 1])

        def phase_pt(u):
            """DVE p*t (ready early, emitted before psum-dependent work)."""
            p16, t16 = st[u]["p16"], st[u]["t16"]
            pt = pt_pool.tile([128, HC * W], F16, tag="pt")
            nc.vector.tensor_tensor(pt[:], _flat(p16[:]), _flat(t16[:]),
                                    mybir.AluOpType.mult)
            st[u]["pt"] = pt

        def phase_ln(u):
            """ACT softplus accumulate (block end, after the decodes)."""
            e = st[u]["e"]
            nc.scalar.activation(e[:], e[:],
                                 mybir.ActivationFunctionType.Ln,
                                 bias=1.0, accum_out=accs[:, u:u + 1])

        def _mm_chunk(u, j, s_tile):
            """Accumulating banded matmuls for chunk j of unit u."""
            n, h = u // 2, u % 2
            c = h * HC + j
            kl_ = [(kd, c)]
            if c > 0:
                kl_.append((ku, c - 1))
            if c < 2 * HC - 1:
                kl_.append((kl, c + 1))
            for q, (kmat, srcc) in enumerate(kl_):
                src_u = n * 2 + srcc // HC
                src_j = srcc % HC
                t_src = st[src_u]["t16"]
                for wh in range(2):
                    ws = slice(wh * 512, (wh + 1) * 512)
                    nc.tensor.matmul(s_tile[:, ws], kmat[:],
                                     t_src[:, src_j, ws],
                                     start=(q == 0),
                                     stop=(q == len(kl_) - 1))

        def _decode_chunk(d16, j, s_tile, on_act):
            """d16[:,j,:] = int16(-hi/512 + 32), hi = S high halfword."""
            b = s_tile[:].bitcast(I16)
            hi = AP(b.tensor, b.offset + 1, [list(b.ap[0]), [2, W]])
            if on_act:
                nc.scalar.activation(d16[:, j, :], hi,
                                     mybir.ActivationFunctionType.Copy,
                                     bias=32.0, scale=-1.0 / 512.0)
            else:
                nc.vector.tensor_scalar(d16[:, j, :], hi, -1.0 / 512.0,
                                        32.0, mybir.AluOpType.mult,
                                        mybir.AluOpType.add)

        def phase_b(u):
            """Banded matmuls -> S psum -> integer distance.
            Decode alternates ACT (chunks 0,2) / DVE (chunks 1,3): ACT
            reaches its decodes later in its queue (after Exp), so it
            gets the earlier-produced psum chunks."""
            d16 = d_pool.tile([128, HC, W], I16, tag="d")
            for j in range(HC):
                s_tile = ps_pool.tile([128, W], F32, tag="s",
                                      name=f"s_{u}_{j}")
                _mm_chunk(u, j, s_tile)
                _decode_chunk(d16, j, s_tile, on_act=(j % 2 == 0))
            st[u]["d16"] = d16

        def phase_md(u, half=None):
            """DVE fused-mask product and count indicator (lagged one
            block so the decodes are always complete)."""
            d16, m = st[u]["d16"], st[u]["m"]
            if "md" not in st[u]:
                st[u]["md"] = md_pool.tile([128, HC * W], F16, tag="md",
                                           name=f"md_{u}")
                st[u]["c01"] = c_pool.tile([128, HC * W], F16, tag="c01",
                                           name=f"c01_{u}")
            md, c01 = st[u]["md"], st[u]["c01"]
            segs = [slice(0, HC * W)] if half is None else [
                slice(half * HC * W // 2, (half + 1) * HC * W // 2)]
            for seg in segs:
                nc.vector.tensor_tensor(md[:, seg], m[:, seg],
                                        _flat(d16[:])[:, seg],
                                        mybir.AluOpType.mult)
                nc.vector.tensor_scalar(c01[:, seg], md[:, seg], 0.0, None,
                                        mybir.AluOpType.is_gt)

        # software-pipelined emission, per-engine queue order per block k:
        #   gpsimd: loads(k+2)
        #   ACT: E(k+1), dec1(k), dec3(k), Ln(k)
        #   DVE: m(k+1), pt(k), dec0(k), dec2(k), md(k-1), c01(k-1)
        #   PE: banded(k), sums(k-2)
        phase_a(0)
        phase_a(1)
        phase_a(2)
        phase_e(0)
        for k in range(NU):
            if k + 3 < NU:
                phase_a(k + 3)
            if k + 1 < NU:
                phase_e(k + 1)
            phase_pt(k)
            phase_b(k)
            if k >= 2:
                phase_sums(k - 2)
            if k >= 1:
                phase_md(k - 1)
            phase_ln(k)
        # tail: finish the last unit at half granularity to shorten the
        # critical chain (PE runs sums while DVE products finish); the
        # psum accumulators drain into acc row 0 right after their stop
        # matmuls, interleaved with the remaining PE sums
        phase_md(NU - 1, half=0)
        phase_sums(NU - 2)
        phase_md(NU - 1, half=1)
        # softplus columns are final once Ln(NU-1) has accumulated
        nc.sync.dma_start(acc_ext[:, 0:8], accs[:, 0:8])
        phase_sums(NU - 1, half=0)
        phase_sums(NU - 1, half=1, reduces=True)

        nc.sync.dma_start(acc_ext[:, 8:11], accs[:, 8:11])

    nc.compile()
    return nc


def _get_nc():
    global _CACHED_NC
    if _CACHED_NC is None:
        _CACHED_NC = _build_nc()
    return _CACHED_NC


def _run(predictions, targets, trace=False, **trace_kwargs):
    """Run the SPMD kernel; returns (loss_scalar, BassKernelResults)."""
    p = np.ascontiguousarray(
        np.asarray(predictions, dtype=np.float32).reshape(32, H, W))
    t = np.ascontiguousarray(
        np.asarray(targets, dtype=np.float32).reshape(32, H, W))
    kd, ku, kl = _k_blocks()

    in_maps = []
    for c in range(N_CORES):
        sl = slice(c * N_PER_CORE, (c + 1) * N_PER_CORE)
        in_maps.append({
            "predictions": np.ascontiguousarray(p[sl]),
            "targets": np.ascontiguousarray(t[sl]),
            "kd": kd, "ku": ku, "kl": kl,
        })

    nc = _get_nc()
    res = run_bass_kernel_spmd(nc, in_maps, list(range(N_CORES)),
                               trace=trace, **trace_kwargs)

    sum_sp = sum_pt = sum_md = cnt = 0.0
    for c in range(N_CORES):
        acc = np.asarray(res.results[c]["acc"], dtype=np.float64)
        sum_sp += acc[:, 0:8].sum()
        sum_pt += acc[0, 8]
        sum_md += acc[0, 9]
        cnt += acc[0, 10]

    n_elem = 32.0 * H * W
    bce = (sum_sp - sum_pt) / n_elem
    border = 0.0 if sum_md == 0.0 else sum_md / max(cnt, 1.0)
    loss = bce + np.sqrt(max(border, 0.0))
    return np.float32(loss), res


def kernel(predictions, targets):
    loss, _ = _run(predictions, targets)
    return np.asarray(loss, dtype=np.float32)


# revision 31
# speedup vs baseline: 1.1273x; 1.0567x over previous
"""DistanceTransformLoss on 8 Trainium2 NeuronCores (Bass/Tile).

loss = BCEWithLogits(predictions, targets).mean()
       + sqrt( sum(pen) / max(count(pen != 0), 1) ),
  pen = (sigmoid(pred) > 0.5) * grassfire_dist_H(targets)

Distance via matmul in natural layout (h on partitions):
  S[i,w] = sum_j q^|i-j| * t[j,w] with q = 2^-4 (banded bf16 matmuls);
  S = 2^(-4*D) * rho, rho in [1, 2.14), so the exact column distance D
  is recovered from the f32 exponent field: d16 = int16(-hi/512 + 32),
  hi = high halfword of S; the value lands in (D - 0.25, D + 0.25].

v4 engine assignment (the accumulate-capable DVE ops all run in 1x
perf mode; only plain tensor_scalar (4x) and tensor_tensor (2x) are
fast, so sums go to ACT-accumulate and PE ones-matmuls):
  - DMA (one SWDGE queue): p and t cast f32->bf16 during the DMA,
    one 2MB-read transfer per tensor per half-image unit
  - ACT: e = Exp(p); softplus Ln(e+1) in-place accumulate -> sum_sp;
    decode of S chunks 0,2 per unit
  - DVE: m = (p>0) TS 4x; pt = p*t TT 2x; md = m*d TT 2x;
    c01 = (md>0) TS 4x; decode of S chunks 1,3 per unit
  - PE:  banded S-matmuls + ones-matmul column sums of pt/md/c01 into
    three persistent [1,512] psum accumulators
Host (f64): bce = (sum_sp - sum_pt)/NEL; border = sum_md/max(cnt,1);
  loss = bce + sqrt(border).

Sharding: data-parallel over batch N (32 images -> 4 per core).
"""
import sys

if "/opt/trn_rl_repo" not in sys.path:
    sys.path.insert(0, "/opt/trn_rl_repo")

import numpy as np
from contextlib import ExitStack

import concourse.bass as bass
import concourse.bacc as bacc
import concourse.tile as tile
from concourse import mybir
from concourse.ap import AP
from concourse.bass_utils import run_bass_kernel_spmd
from concourse.hw_specs import get_activation_tables

N_CORES = 8
N_PER_CORE = 4          # 32 images / 8 cores
H = 1024
W = 1024
HC = 4                  # 128-row chunks per half-image unit
NU = 2 * N_PER_CORE     # pipeline units (half-images) per core

F32 = mybir.dt.float32
F16 = mybir.dt.float16
BF16 = mybir.dt.bfloat16
I16 = mybir.dt.int16

# acc layout: [128, 11] f32 columns:
#   [0:8)  softplus sums per unit (ACT accumulate)
#   [8]    sum p*t   (row 0; psum cache-reduce)
#   [9]    sum m*d   (row 0)
#   [10]   count     (row 0)
ACC_COLS = 11

_CACHED_NC = None


def _flat(ap):
    """Flatten the free dims of a contiguous [128, ...] AP to [128, F]."""
    (pstep, pcount) = ap.ap[0]
    f = 1
    for (_, c) in ap.ap[1:]:
        f *= c
    return AP(ap.tensor, ap.offset, [[pstep, pcount], [1, f]])


def _k_blocks():
    """The three constant kernel blocks [j, i] in bf16, q = 2^-4.

    KD[j,i] = q^|i-j|, KU[j,i] = q^(128+i-j), KL[j,i] = q^(128+j-i).
    All entries are exact powers of two (zero beyond distance 31), so
    S = sum_j K[i,j] t[j] = 2^(-4D) * rho with rho in [1, 2.14) and the
    distance D is recovered exactly from the f32 exponent field of S.
    Host-precast to bf16 so they load over the idle HWDGE queue.
    """
    import ml_dtypes
    j = np.arange(128, dtype=np.float64)[:, None]
    i = np.arange(128, dtype=np.float64)[None, :]
    out = []
    for dmat in (np.abs(i - j), 128.0 + i - j, 128.0 + j - i):
        k = np.where(dmat <= 31, np.power(2.0, -4.0 * dmat), 0.0)
        out.append(np.ascontiguousarray(k.astype(ml_dtypes.bfloat16)))
    return out


def _unit_src(ext, n, h):
    """DRAM AP for one half-image in chunk-major [128, HC, W] order."""
    off = n * H * W + h * (H // 2) * W
    return AP(ext.tensor, off, [[W, 128], [128 * W, HC], [1, W]])


def _build_nc():
    nc = bacc.Bacc("TRN2", target_bir_lowering=False, debug=False,
                   enable_asserts=False)
    t_ext = nc.dram_tensor("targets", [N_PER_CORE, H, W], F32,
                           kind="ExternalInput").ap()
    p_ext = nc.dram_tensor("predictions", [N_PER_CORE, H, W], F32,
                           kind="ExternalInput").ap()
    kd_ext = nc.dram_tensor("kd", [128, 128], BF16, kind="ExternalInput").ap()
    ku_ext = nc.dram_tensor("ku", [128, 128], BF16, kind="ExternalInput").ap()
    kl_ext = nc.dram_tensor("kl", [128, 128], BF16, kind="ExternalInput").ap()
    acc_ext = nc.dram_tensor("acc", [128, ACC_COLS], F32,
                             kind="ExternalOutput").ap()

    with tile.TileContext(nc) as tc, ExitStack() as ctx:
        const_pool = ctx.enter_context(tc.tile_pool(name="const", bufs=1))
        p_pool = ctx.enter_context(tc.tile_pool(name="p16", bufs=5))
        t_pool = ctx.enter_context(tc.tile_pool(name="t16", bufs=5))
        e_pool = ctx.enter_context(tc.tile_pool(name="e", bufs=2))
        m_pool = ctx.enter_context(tc.tile_pool(name="m", bufs=3))
        d_pool = ctx.enter_context(tc.tile_pool(name="d16", bufs=2))
        md_pool = ctx.enter_context(tc.tile_pool(name="md", bufs=2))
        pt_pool = ctx.enter_context(tc.tile_pool(name="pt", bufs=3))
        c_pool = ctx.enter_context(tc.tile_pool(name="c01", bufs=2))
        acc_pool = ctx.enter_context(tc.tile_pool(name="acc", bufs=1))
        ps_pool = ctx.enter_context(tc.tile_pool(name="ps", bufs=3,
                                                 space="PSUM"))
        psacc_pool = ctx.enter_context(tc.tile_pool(name="psacc", bufs=1,
                                                    space="PSUM"))

        # Pre-load the act table containing BOTH Exp and Ln.
        tables = list(get_activation_tables(nc.m.arch).items())
        set_id = next(i for i, (_, fns) in enumerate(tables)
                      if mybir.ActivationFunctionType.Exp in fns
                      and mybir.ActivationFunctionType.Ln in fns)
        nc.scalar.add_instruction(mybir.InstLoadActFuncSet(
            name=nc.get_next_instruction_name(),
            act_func_set_id=set_id, ins=[], outs=[]))

        # constants: kernel blocks (host-precast bf16, HWDGE queue so the
        # gpsimd SWDGE queue starts on the first input transfer at t=0)
        kd = const_pool.tile([128, 128], BF16, tag="kd")
        ku = const_pool.tile([128, 128], BF16, tag="ku")
        kl = const_pool.tile([128, 128], BF16, tag="kl")
        nc.sync.dma_start(kd[:], kd_ext)
        nc.sync.dma_start(ku[:], ku_ext)
        nc.sync.dma_start(kl[:], kl_ext)

        accs = acc_pool.tile([128, ACC_COLS], F32)
        nc.vector.memset(accs[:], 0.0)
        ones_f16 = const_pool.tile([128, 1], F16, tag="ones_f16")
        nc.gpsimd.memset(ones_f16[:], 1.0)

        # persistent PE reduction accumulators in one psum bank
        # (matmul out base partition must be 0/32/64)
        ps_all = psacc_pool.tile([65, 512], F32, tag="ps_all")
        ps_pt = ps_all[0:1, :]
        ps_md = ps_all[32:33, :]
        ps_c = ps_all[64:65, :]

        st = [dict() for _ in range(NU)]

        def phase_a(u):
            """SWDGE cast loads: one 2MB-read transfer per tensor."""
            n, h = u // 2, u % 2
            t16 = t_pool.tile([128, HC, W], BF16, tag="t16")
            p16 = p_pool.tile([128, HC, W], BF16, tag="p16")
            if u >= NU - 2 or u <= 1:
                # head: Exp can start early; tail: BCE chain finishes early
                nc.gpsimd.dma_start(p16[:], _unit_src(p_ext, n, h))
                nc.gpsimd.dma_start(t16[:], _unit_src(t_ext, n, h))
            else:
                nc.gpsimd.dma_start(t16[:], _unit_src(t_ext, n, h))
                nc.gpsimd.dma_start(p16[:], _unit_src(p_ext, n, h))
            st[u]["t16"] = t16
            st[u]["p16"] = p16

        def phase_e(u):
            """ACT: e = Exp(p).  DVE: m = (p > 0) in 4x mode."""
            p16 = st[u]["p16"]
            e = e_pool.tile([128, HC * W], F16, tag="e")
            nc.scalar.activation(e[:], _flat(p16[:]),
                                 mybir.ActivationFunctionType.Exp)
            m = m_pool.tile([128, HC * W], BF16, tag="m")
            nc.vector.tensor_scalar(m[:], _flat(p16[:]), 0.0, None,
                                    mybir.AluOpType.is_gt)
            st[u]["e"] = e
            st[u]["m"] = m

        def phase_sums(u, half=None, reduces=None):
            """PE ones-matmul column sums of pt/md/c01 (prev unit).
            `reduces`: accs columns to drain each accumulator into right
            after its stop matmul (tail interleaving)."""
            first, last = (u == 0), (u == NU - 1)
            pt, md, c01 = st[u]["pt"], st[u]["md"], st[u]["c01"]
            segs = range(8) if half is None else range(4 * half, 4 * half + 4)
            for dst, src, col in ((ps_pt, pt, 8), (ps_md, md, 9),
                                  (ps_c, c01, 10)):
                for s in segs:
                    ws = slice(s * 512, (s + 1) * 512)
                    nc.tensor.matmul(dst, ones_f16[:], src[:, ws],
                                     start=(first and s == 0),
                                     stop=(last and s == 7))
                if reduces is not None and 7 in segs:
                    nc.vector.tensor_scalar(
                        dst, dst, 1.0, 0.0, mybir.AluOpType.mult,
                        mybir.AluOpType.add,
                        accum_out=accs[0:1, col:col + 1])

        def phase_pt(u):
            """DVE p*t (ready early, emitted before psum-dependent work)."""
            p16, t16 = st[u]["p16"], st[u]["t16"]
            pt = pt_pool.tile([128, HC * W], F16, tag="pt")
            nc.vector.tensor_tensor(pt[:], _flat(p16[:]), _flat(t16[:]),
                                    mybir.AluOpType.mult)
            st[u]["pt"] = pt

        def phase_ln(u):
            """ACT softplus accumulate (block end, after the decodes)."""
            e = st[u]["e"]
            nc.scalar.activation(e[:], e[:],
                                 mybir.ActivationFunctionType.Ln,
                                 bias=1.0, accum_out=accs[:, u:u + 1])

        def _mm_chunk(u, j, s_tile):
            """Accumulating banded matmuls for chunk j of unit u."""
            n, h = u // 2, u % 2
            c = h * HC + j
            kl_ = [(kd, c)]
            if c > 0:
                kl_.append((ku, c - 1))
            if c < 2 * HC - 1:
                kl_.append((kl, c + 1))
            for q, (kmat, srcc) in enumerate(kl_):
                src_u = n * 2 + srcc // HC
                src_j = srcc % HC
                t_src = st[src_u]["t16"]
                for wh in range(2):
                    ws = slice(wh * 512, (wh + 1) * 512)
                    nc.tensor.matmul(s_tile[:, ws], kmat[:],
                                     t_src[:, src_j, ws],
                                     start=(q == 0),
                                     stop=(q == len(kl_) - 1))

        def _decode_chunk(d16, j, s_tile, on_act):
            """d16[:,j,:] = int16(-hi/512 + 32), hi = S high halfword."""
            b = s_tile[:].bitcast(I16)
            hi = AP(b.tensor, b.offset + 1, [list(b.ap[0]), [2, W]])
            if on_act:
                nc.scalar.activation(d16[:, j, :], hi,
                                     mybir.ActivationFunctionType.Copy,
                                     bias=32.0, scale=-1.0 / 512.0)
            else:
                nc.vector.tensor_scalar(d16[:, j, :], hi, -1.0 / 512.0,
                                        32.0, mybir.AluOpType.mult,
                                        mybir.AluOpType.add)

        def phase_b(u):
            """Banded matmuls -> S psum -> integer distance.
            Decode alternates DVE (chunks 0,2) / ACT (chunks 1,3) in psum
            production order so tiles free steadily for the next unit."""
            d16 = d_pool.tile([128, HC, W], I16, tag="d")
            for j in range(HC):
                s_tile = ps_pool.tile([128, W], F32, tag="s",
                                      name=f"s_{u}_{j}")
                _mm_chunk(u, j, s_tile)
                _decode_chunk(d16, j, s_tile, on_act=(j % 2 == 1))
            st[u]["d16"] = d16

        def phase_md(u, half=None):
            """DVE fused-mask product and count indicator (lagged one
            block so the decodes are always complete)."""
            d16, m = st[u]["d16"], st[u]["m"]
            if "md" not in st[u]:
                st[u]["md"] = md_pool.tile([128, HC * W], F16, tag="md",
                                           name=f"md_{u}")
                st[u]["c01"] = c_pool.tile([128, HC * W], F16, tag="c01",
                                           name=f"c01_{u}")
            md, c01 = st[u]["md"], st[u]["c01"]
            segs = [slice(0, HC * W)] if half is None else [
                slice(half * HC * W // 2, (half + 1) * HC * W // 2)]
            for seg in segs:
                nc.vector.tensor_tensor(md[:, seg], m[:, seg],
                                        _flat(d16[:])[:, seg],
                                        mybir.AluOpType.mult)
                nc.vector.tensor_scalar(c01[:, seg], md[:, seg], 0.0, None,
                                        mybir.AluOpType.is_gt)

        # software-pipelined emission, per-engine queue order per block k:
        #   gpsimd: loads(k+2)
        #   ACT: E(k+1), dec1(k), dec3(k), Ln(k)
        #   DVE: m(k+1), pt(k), dec0(k), dec2(k), md(k-1), c01(k-1)
        #   PE: banded(k), sums(k-2)
        phase_a(0)
        phase_a(1)
        phase_a(2)
        phase_e(0)
        for k in range(NU):
            if k + 3 < NU:
                phase_a(k + 3)
            if k + 1 < NU:
                phase_e(k + 1)
            phase_pt(k)
            phase_b(k)
            if k >= 2:
                phase_sums(k - 2)
            if k >= 1:
                phase_md(k - 1)
            phase_ln(k)
        # tail: finish the last unit at half granularity to shorten the
        # critical chain (PE runs sums while DVE products finish); the
        # psum accumulators drain into acc row 0 right after their stop
        # matmuls, interleaved with the remaining PE sums
        phase_md(NU - 1, half=0)
        phase_sums(NU - 2)
        phase_md(NU - 1, half=1)
        # softplus columns are final once Ln(NU-1) has accumulated
        nc.sync.dma_start(acc_ext[:, 0:8], accs[:, 0:8])
        phase_sums(NU - 1, half=0)
        phase_sums(NU - 1, half=1, reduces=True)

        nc.sync.dma_start(acc_ext[:, 8:11], accs[:, 8:11])

    nc.compile()
    return nc


def _get_nc():
    global _CACHED_NC
    if _CACHED_NC is None:
        _CACHED_NC = _build_nc()
    return _CACHED_NC


def _run(predictions, targets, trace=False, **trace_kwargs):
    """Run the SPMD kernel; returns (loss_scalar, BassKernelResults)."""
    p = np.ascontiguousarray(
        np.asarray(predictions, dtype=np.float32).reshape(32, H, W))
    t = np.ascontiguousarray(
        np.asarray(targets, dtype=np.float32).reshape(32, H, W))
    kd, ku, kl = _k_blocks()

    in_maps = []
    for c in range(N_CORES):
        sl = slice(c * N_PER_CORE, (c + 1) * N_PER_CORE)
        in_maps.append({
            "predictions": np.ascontiguousarray(p[sl]),
            "targets": np.ascontiguousarray(t[sl]),
            "kd": kd, "ku": ku, "kl": kl,
        })

    nc = _get_nc()
    res = run_bass_kernel_spmd(nc, in_maps, list(range(N_CORES)),
                               trace=trace, **trace_kwargs)

    sum_sp = sum_pt = sum_md = cnt = 0.0
    for c in range(N_CORES):
        acc = np.asarray(res.results[c]["acc"], dtype=np.float64)
        sum_sp += acc[:, 0:8].sum()
        sum_pt += acc[0, 8]
        sum_md += acc[0, 9]
        cnt += acc[0, 10]

    n_elem = 32.0 * H * W
    bce = (sum_sp - sum_pt) / n_elem
    border = 0.0 if sum_md == 0.0 else sum_md / max(cnt, 1.0)
    loss = bce + np.sqrt(max(border, 0.0))
    return np.float32(loss), res


def kernel(predictions, targets):
    loss, _ = _run(predictions, targets)
    return np.asarray(loss, dtype=np.float32)
